# revision 2
# baseline (speedup 1.0000x reference)
"""AttentionHead kernel for 8 Trainium2 NeuronCores (SPMD data-parallel).

Problem: q/k/v projections [1024->64] + masked softmax attention,
B=4, S=2048, d_model=1024, d_k=64.

Sharding: 8 cores = 4 batches x 2 query-halves. Each core handles one
(batch, q-half): query shard [1024, 1024], full key/value for its batch
[2048, 1024], mask shard [1024, 2048]. Weights replicated.

Per-core device pipeline (everything contracts on the partition dim;
all inputs host-packed so each DMA is one large contiguous transfer,
ordered just-in-time for the compute pipeline):
  - warm-up matmuls on a memset junk tile (no DMA dependency) keep the
    PE HAM clock ramping from t~1us; they accumulate garbage into the
    PV psum region, which PV's start=True later clears
  - projections are col-tiled: each m-block matmul is emitted twice at
    column positions 0 and 64, producing qT/kT duplicated across both
    partition halves (the pair runs concurrently in the array)
  - scores are computed TRANSPOSED [skv_tile=128, sq] and row-tiled:
    tiles j (rows 0-63) and j+1 (rows 64-127) run concurrently since
    the contraction (d_k) is only 64
  - ACT exp reads scores straight from PSUM with the 1024**-0.5 scale
    fused; masked positions then set to 1.0 (==exp(1e-9) in fp32) via
    copy_predicated on the bf16 E pair tile
  - PV accumulates transposed: oT[65, sq] += vaug_j.T @ E_j; row 64 is
    the softmax denominator (free via the ones column of vaug)
  - finalize: single psum->sbuf copy of oT [65, sq] + one output DMA;
    the transpose back to [sq, 64] and the division by the denominator
    run on the host during unshard
"""

import numpy as np
import ml_dtypes

B = 4
S = 2048
D_MODEL = 1024
D_K = 64
N_CORES = 8

P = 128
SQ = S // 2          # per-core query rows (1024)
SKV = S              # per-core kv rows (2048)
MB = D_MODEL // P    # 8 m-blocks (contraction)
JT = SKV // P        # 16 skv tiles
NQC = SQ // 512      # 2 q chunks
NKC = SKV // 512     # 4 k chunks
NPAIR = JT // 2      # 8 skv tile pairs
VG = 8               # v DMA groups (2 skv tiles each)
JPG = JT // VG       # skv tiles per v group (2)

_BF16 = ml_dtypes.bfloat16

_cached_nc = None


def _build_nc():
    import concourse.mybir as mybir
    import concourse.tile as tile
    from concourse import bacc

    bf16 = mybir.dt.bfloat16
    f32 = mybir.dt.float32
    u8 = mybir.dt.uint8

    nc = bacc.Bacc(None, target_bir_lowering=False)

    w_d = nc.dram_tensor("w_all", [P, MB, 3 * D_K], bf16, kind="ExternalInput")
    q_d = nc.dram_tensor("q_t", [P, NQC, MB, 512], bf16, kind="ExternalInput")
    k_d = nc.dram_tensor("k_t", [P, NKC, MB, 512], bf16, kind="ExternalInput")
    v_d = nc.dram_tensor("v_t", [P, VG, JPG, MB, P], bf16, kind="ExternalInput")
    m_d = nc.dram_tensor("mask_t", [P, NPAIR, 2, SQ], u8, kind="ExternalInput")
    out_d = nc.dram_tensor("out", [D_K + 1, SQ], f32, kind="ExternalOutput")

    with tile.TileContext(nc) as tc:
        with (
            tc.tile_pool(name="const", bufs=1) as cpool,
            tc.tile_pool(name="inp", bufs=1) as ipool,
            tc.tile_pool(name="proj", bufs=1) as jpool,
            tc.tile_pool(name="ps_s", bufs=4, space="PSUM") as ps_s,
            tc.tile_pool(name="ps_pv", bufs=2, space="PSUM") as ps_pv,
            tc.tile_pool(name="ps_o", bufs=1, space="PSUM") as ps_o,
        ):
            # ---- junk tile for DMA-independent PE warm-up ----
            junk = cpool.tile([P, 512], bf16, tag="junk")
            nc.vector.memset(junk, 0.125)

            # ---- input DMAs: emission order == stream order (HWDGE
            # FIFO); ordered so each tensor lands just before its
            # consumer is ready ----
            w_sb = cpool.tile([P, MB, 3 * D_K], bf16, tag="w")
            nc.sync.dma_start(out=w_sb, in_=w_d[:])

            qcs = [None] * NQC
            kcs = [None] * NKC
            mps = [None] * NPAIR
            vgs = [None] * VG

            def dma_q(t):
                qc = ipool.tile([P, MB, 512], bf16, tag=f"q{t}", name=f"q{t}")
                nc.sync.dma_start(out=qc, in_=q_d[:, t])
                qcs[t] = qc

            def dma_k(t):
                kc = ipool.tile([P, MB, 512], bf16, tag=f"k{t}", name=f"k{t}")
                nc.sync.dma_start(out=kc, in_=k_d[:, t])
                kcs[t] = kc

            def dma_m(p):
                mp = ipool.tile([P, 2, SQ], u8, tag=f"m{p}", name=f"m{p}")
                nc.sync.dma_start(out=mp, in_=m_d[:, p])
                mps[p] = mp

            def dma_v(g):
                vg = ipool.tile([P, JPG, MB, P], bf16, tag=f"v{g}", name=f"v{g}")
                nc.sync.dma_start(out=vg, in_=v_d[:, g])
                vgs[g] = vg

            dma_k(0)
            dma_q(0)
            dma_m(0)
            dma_q(1)
            dma_v(0)
            dma_k(1)
            dma_m(1)
            dma_v(1)
            dma_m(2)
            dma_v(2)
            dma_k(2)
            dma_m(3)
            dma_v(3)
            dma_m(4)
            dma_v(4)
            dma_k(3)
            dma_m(5)
            dma_v(5)
            dma_m(6)
            dma_v(6)
            dma_m(7)
            dma_v(7)

            def wq(i):
                return w_sb[:, i, 0:D_K]

            def wk(i):
                return w_sb[:, i, D_K : 2 * D_K]

            def wv(i):
                return w_sb[:, i, 2 * D_K : 3 * D_K]

            # ---- constants ----
            ones_bf = cpool.tile([P, 2, SQ], bf16, tag="ones")
            nc.vector.memset(ones_bf, 1.0)

            # ---- transposed output accumulator [65, 1024] f32 = 2
            # psum banks; one accumulation group per 512-col bank ----
            oTp = ps_o.tile([D_K + 1, SQ], f32, tag="oT")

            # ---- PE warm-up: junk matmuls bridge the gap until k0
            # lands, ramping the HAM clock; they accumulate into oTp,
            # which PV's start=True clears later ----
            for wi in range(10):
                nc.tensor.matmul(
                    oTp[0:D_K, 0:512],
                    lhsT=junk[:, 0:D_K],
                    rhs=junk,
                    start=(wi == 0),
                    stop=(wi == 9),
                )

            # ---- projections: col-tiled so the projected tensor is
            # duplicated across both partition halves (row-tiled scores
            # need lhsT/rhs at both base partitions 0 and 64) ----
            qTd = jpool.tile([P, SQ], bf16, tag="qT")
            kTd = jpool.tile([P, SKV], bf16, tag="kT")

            def proj_dup(w_of, rhs_chunk):
                """[128, 512] psum with identical [64, 512] projections
                in both partition halves (pairwise-concurrent MMs)."""
                pp = ps_s.tile([P, 512], f32, tag="sp")
                for i in range(MB):
                    for h in (0, 1):
                        nc.tensor.matmul(
                            pp[h * D_K : (h + 1) * D_K, :],
                            lhsT=w_of(i),
                            rhs=rhs_chunk(i),
                            start=(i == 0),
                            stop=(i == MB - 1),
                            tile_position=(0, h * D_K),
                        )
                return pp

            def kproj(t):
                pp = proj_dup(wk, lambda i: kcs[t][:, i, :])
                # psum f32 -> sbuf bf16 on ACT (keeps DVE for the mask)
                nc.scalar.copy(kTd[:, t * 512 : (t + 1) * 512], pp)

            def qproj(t):
                pp = proj_dup(wq, lambda i: qcs[t][:, i, :])
                nc.scalar.copy(qTd[:, t * 512 : (t + 1) * 512], pp)

            kproj(0)
            qproj(0)
            qproj(1)

            # ---- per-j v-aug tiles (col 64 = ones) ----
            vaugs = []
            for j in range(JT):
                va = jpool.tile([P, D_K + 1], bf16, tag=f"va{j}", name=f"va{j}")
                nc.vector.memset(va[:, D_K : D_K + 1], 1.0)
                vaugs.append(va)

            # ---- per-pair E tiles [128, 2, 1024] ----
            Eps = [
                jpool.tile([P, 2, SQ], bf16, tag=f"E{p}", name=f"E{p}")
                for p in range(NPAIR)
            ]

            def E_of(j):
                return Eps[j // 2][:, j % 2, :]

            # ---- main pipeline over skv tile pairs.
            # Emission order is engine-queue order. Stagger: vproj one
            # pair late, PV two pairs late, mask-pred one pair late.
            pvs = [None] * JT

            def emit_vproj(j):
                g, jj = divmod(j, JPG)
                pv = ps_pv.tile([P, D_K], f32, tag="pv", name=f"pv{j}")
                for i in range(MB):
                    nc.tensor.matmul(
                        pv,
                        lhsT=vgs[g][:, jj, i, :],
                        rhs=wv(i),
                        start=(i == 0),
                        stop=(i == MB - 1),
                    )
                pvs[j] = pv

            def emit_cast(j):
                nc.vector.tensor_copy(vaugs[j][:, 0:D_K], pvs[j])

            def emit_pv(j):
                for c in range(NQC):
                    nc.tensor.matmul(
                        oTp[:, c * 512 : (c + 1) * 512],
                        lhsT=vaugs[j],
                        rhs=E_of(j)[:, c * 512 : (c + 1) * 512],
                        start=(j == 0),
                        stop=(j == JT - 1),
                    )

            def emit_pred(p):
                nc.vector.copy_predicated(
                    out=Eps[p], mask=mps[p], data=ones_bf
                )

            def emit_scores(p):
                # row-tiled: tile j0 at array rows 0-63, j1 at 64-127;
                # each (chunk, j) matmul gets its own psum bank
                j0, j1 = 2 * p, 2 * p + 1
                for c in range(NQC):
                    csl = slice(c * 512, (c + 1) * 512)
                    for h, j in ((0, j0), (1, j1)):
                        sp = ps_s.tile([P, 512], f32, tag="sp", name=f"s{j}c{c}")
                        nc.tensor.matmul(
                            sp,
                            lhsT=kTd[h * D_K : (h + 1) * D_K, j * P : (j + 1) * P],
                            rhs=qTd[h * D_K : (h + 1) * D_K, csl],
                            start=True,
                            stop=True,
                            tile_position=(h * D_K, 0),
                        )
                        # E = exp(s / sqrt(d_model)) straight from PSUM
                        nc.scalar.activation(
                            out=E_of(j)[:, csl],
                            in_=sp,
                            func=mybir.ActivationFunctionType.Exp,
                            scale=float(D_MODEL) ** -0.5,
                        )

            for p in range(NPAIR):
                emit_scores(p)
                if p % 2 == 0 and p // 2 + 1 < NKC:
                    kproj(p // 2 + 1)
                if p >= 1:
                    emit_vproj(2 * p - 2)
                    emit_vproj(2 * p - 1)
                    emit_cast(2 * p - 2)
                    emit_cast(2 * p - 1)
                    emit_pred(p - 1)
                if p >= 2:
                    emit_pv(2 * p - 4)
                    emit_pv(2 * p - 3)

            # drain the staggered tail
            emit_vproj(JT - 2)
            emit_vproj(JT - 1)
            emit_cast(JT - 2)
            emit_cast(JT - 1)
            emit_pred(NPAIR - 1)
            for j in range(JT - 4, JT):
                emit_pv(j)

            # ---- finalize: one psum->sbuf copy + one output DMA;
            # transpose + division happen host-side ----
            oT_sb = jpool.tile([D_K + 1, SQ], f32, tag="oTs")
            nc.scalar.copy(oT_sb, oTp)
            nc.sync.dma_start(out=out_d[:], in_=oT_sb)

    nc.finalize()
    return nc


def _get_nc():
    global _cached_nc
    if _cached_nc is None:
        _cached_nc = _build_nc()
    return _cached_nc


def _shard_inputs(query, key, value, mask, w_q, w_k, w_v):
    """Host-side shard + layout prep. Core c -> (batch c//2, q-half c%2)."""
    w_all = np.concatenate(
        [
            w.T.astype(_BF16).reshape(MB, P, D_K).transpose(1, 0, 2)
            for w in (w_q, w_k, w_v)
        ],
        axis=2,
    )
    w_all = np.ascontiguousarray(w_all)
    in_maps = []
    for c in range(N_CORES):
        b, h = divmod(c, 2)
        s0 = h * SQ
        q_t = query[b, s0 : s0 + SQ, :].T.astype(_BF16)   # [m, sq]
        k_t = key[b].T.astype(_BF16)                      # [m, skv]
        v_t = value[b].T.astype(_BF16)                    # [m, skv]
        m_t = mask[b, s0 : s0 + SQ, :].T.astype(np.uint8)  # [skv, sq]
        in_maps.append(
            {
                "w_all": w_all,
                # [m, sq] -> [128, NQC, MB, 512]
                "q_t": np.ascontiguousarray(
                    q_t.reshape(MB, P, NQC, 512).transpose(1, 2, 0, 3)
                ),
                # [m, skv] -> [128, NKC, MB, 512]
                "k_t": np.ascontiguousarray(
                    k_t.reshape(MB, P, NKC, 512).transpose(1, 2, 0, 3)
                ),
                # [m, skv] -> [128, VG, JPG, MB, 128]
                "v_t": np.ascontiguousarray(
                    v_t.reshape(MB, P, VG, JPG, P).transpose(1, 2, 3, 0, 4)
                ),
                # [skv, sq] -> [128, NPAIR, 2, SQ]
                "mask_t": np.ascontiguousarray(
                    m_t.reshape(NPAIR, 2, P, SQ).transpose(2, 0, 1, 3)
                ),
            }
        )
    return in_maps


def run(inputs, trace=False):
    """Run the SPMD kernel; returns (output [B,S,D_K] f32, BassKernelResults)."""
    from concourse.bass_utils import run_bass_kernel_spmd

    nc = _get_nc()
    in_maps = _shard_inputs(**inputs)
    res = run_bass_kernel_spmd(
        nc, in_maps, core_ids=list(range(N_CORES)), trace=trace
    )
    out = np.empty((B, S, D_K), np.float32)
    for c in range(N_CORES):
        b, h = divmod(c, 2)
        # device out is oT [65, SQ]: rows 0-63 = numerator.T, row 64 =
        # softmax denominator
        oT = res.results[c]["out"]
        out[b, h * SQ : (h + 1) * SQ, :] = (oT[0:D_K] / oT[D_K : D_K + 1]).T
    return out, res


def kernel(**inputs):
    out, _ = run(inputs, trace=False)
    return out


# revision 4
# speedup vs baseline: 1.2500x; 1.2500x over previous
"""AttentionHead kernel for 8 Trainium2 NeuronCores (SPMD data-parallel).

Problem: q/k/v projections [1024->64] + masked softmax attention,
B=4, S=2048, d_model=1024, d_k=64.

Sharding: 8 cores = 4 batches x 2 query-halves. Each core handles one
(batch, q-half): query shard [1024, 1024], full key/value for its batch
[2048, 1024], mask shard [1024, 2048]. Weights replicated.

Precision: raw query/key and w_q/w_k ship as fp8e4m3 (halves their DMA
bytes; scores only need ~2 decimal digits since they sit in an exp with
|s| < ~2). value/w_v stay bf16 (the output is linear in v, so fp8 there
would blow the error budget).

Per-core device pipeline (everything contracts on the partition dim;
all inputs host-packed so each DMA is one large contiguous transfer,
ordered just-in-time for the compute pipeline):
  - warm-up matmuls on a memset junk tile (no DMA dependency) keep the
    PE clock ramping from t~5us; they accumulate garbage into the PV
    psum region, which PV's start=True later clears
  - q/k projection weights are host-duplicated [w|w] so one matmul per
    m-block yields the projection replicated across both partition
    halves (128-col stationary -> fast weight load, no col-tiling)
  - scores are computed TRANSPOSED [skv_tile=128, sq] and row-tiled:
    tiles j (array rows 0-63, from kT/qT partitions 0-63) and j+1
    (rows 64-127, from the duplicated halves) run concurrently since
    the contraction (d_k) is only 64
  - ACT exp reads scores straight from PSUM with the 1024**-0.5 scale
    fused; masked positions then set to 1.0 (==exp(1e-9) in fp32) via
    per-(tile, chunk) copy_predicated [128, 512] on the bf16 E tiles
  - PV accumulates transposed: oT[65, sq] += vaug_j.T @ E_j; row 64 is
    the softmax denominator (free via the ones column of vaug)
  - finalize: per-chunk psum->sbuf copy + output DMA of oT [65, sq];
    the transpose back to [sq, 64] and the division by the denominator
    run on the host during unshard
"""

import numpy as np
import ml_dtypes

B = 4
S = 2048
D_MODEL = 1024
D_K = 64
N_CORES = 8

P = 128
SQ = S // 2          # per-core query rows (1024)
SKV = S              # per-core kv rows (2048)
MB = D_MODEL // P    # 8 m-blocks (contraction)
JT = SKV // P        # 16 skv tiles
NQC = SQ // 512      # 2 q chunks
NKC = SKV // 512     # 4 k chunks
NPAIR = JT // 2      # 8 skv tile pairs
VG = 8               # v DMA groups (2 skv tiles each)
JPG = JT // VG       # skv tiles per v group (2)

_BF16 = ml_dtypes.bfloat16
_FP8 = ml_dtypes.float8_e4m3

_cached_nc = None


def _build_nc():
    import concourse.mybir as mybir
    import concourse.tile as tile
    from concourse import bacc

    bf16 = mybir.dt.bfloat16
    fp8 = mybir.dt.float8e4
    f32 = mybir.dt.float32
    u8 = mybir.dt.uint8

    nc = bacc.Bacc(None, target_bir_lowering=False)

    wqk_d = nc.dram_tensor("w_qk", [P, MB, 4 * D_K], fp8, kind="ExternalInput")
    wv_d = nc.dram_tensor("w_v", [P, MB, D_K], bf16, kind="ExternalInput")
    q_d = nc.dram_tensor("q_t", [P, NQC, MB, 512], fp8, kind="ExternalInput")
    k_d = nc.dram_tensor("k_t", [P, NKC, MB, 512], fp8, kind="ExternalInput")
    v_d = nc.dram_tensor("v_t", [P, VG, JPG, MB, P], bf16, kind="ExternalInput")
    m_d = nc.dram_tensor("mask_t", [P, NPAIR, 2, SQ], u8, kind="ExternalInput")
    out_d = nc.dram_tensor("out", [D_K + 1, SQ], f32, kind="ExternalOutput")

    with tile.TileContext(nc) as tc:
        with (
            tc.tile_pool(name="const", bufs=1) as cpool,
            tc.tile_pool(name="inp", bufs=1) as ipool,
            tc.tile_pool(name="proj", bufs=1) as jpool,
            tc.tile_pool(name="ps_s", bufs=5, space="PSUM") as ps_s,
            tc.tile_pool(name="ps_pv", bufs=1, space="PSUM") as ps_pv,
            tc.tile_pool(name="ps_o", bufs=1, space="PSUM") as ps_o,
        ):
            # ---- junk tile for DMA-independent PE warm-up ----
            junk = cpool.tile([P, 512], bf16, tag="junk")
            nc.vector.memset(junk, 0.125)

            # ---- input DMAs: emission order == stream order (HWDGE
            # FIFO); ordered so each tensor lands just before its
            # consumer is ready ----
            wqk_sb = cpool.tile([P, MB, 4 * D_K], fp8, tag="wqk")
            nc.sync.dma_start(out=wqk_sb, in_=wqk_d[:])
            wv_sb = cpool.tile([P, MB, D_K], bf16, tag="wv")
            nc.sync.dma_start(out=wv_sb, in_=wv_d[:])

            qcs = [None] * NQC
            kcs = [None] * NKC
            mps = [None] * NPAIR
            vgs = [None] * VG

            def dma_q(t):
                qc = ipool.tile([P, MB, 512], fp8, tag=f"q{t}", name=f"q{t}")
                nc.sync.dma_start(out=qc, in_=q_d[:, t])
                qcs[t] = qc

            def dma_k(t):
                kc = ipool.tile([P, MB, 512], fp8, tag=f"k{t}", name=f"k{t}")
                nc.sync.dma_start(out=kc, in_=k_d[:, t])
                kcs[t] = kc

            def dma_m(p):
                mp = ipool.tile([P, 2, SQ], u8, tag=f"m{p}", name=f"m{p}")
                nc.sync.dma_start(out=mp, in_=m_d[:, p])
                mps[p] = mp

            def dma_v(g):
                vg = ipool.tile([P, JPG, MB, P], bf16, tag=f"v{g}", name=f"v{g}")
                nc.sync.dma_start(out=vg, in_=v_d[:, g])
                vgs[g] = vg

            dma_k(0)
            dma_q(0)
            dma_m(0)
            dma_q(1)
            dma_v(0)
            dma_k(1)
            dma_m(1)
            dma_v(1)
            dma_m(2)
            dma_v(2)
            dma_k(2)
            dma_m(3)
            dma_v(3)
            dma_m(4)
            dma_v(4)
            dma_k(3)
            dma_m(5)
            dma_v(5)
            dma_m(6)
            dma_v(6)
            dma_m(7)
            dma_v(7)

            def wqq(i):  # [wq | wq], 128 cols
                return wqk_sb[:, i, 0 : 2 * D_K]

            def wkk(i):  # [wk | wk], 128 cols
                return wqk_sb[:, i, 2 * D_K : 4 * D_K]

            def wv(i):
                return wv_sb[:, i, :]

            # ---- constants ----
            ones_bf = cpool.tile([P, 512], bf16, tag="ones")
            nc.vector.memset(ones_bf, 1.0)

            # ---- transposed output accumulator [65, 1024] f32 = 2
            # psum banks; one accumulation group per 512-col bank ----
            oTp = ps_o.tile([D_K + 1, SQ], f32, tag="oT")

            # ---- PE warm-up: junk matmuls bridge the gap until k0
            # lands, ramping the HAM clock; they accumulate into oTp,
            # which PV's start=True clears later ----
            for wi in range(14):
                nc.tensor.matmul(
                    oTp[0:D_K, 0:512],
                    lhsT=junk[:, 0:D_K],
                    rhs=junk,
                    start=(wi == 0),
                    stop=(wi == 13),
                )

            # ---- projections: host-duplicated weights yield the
            # projected tensor replicated across both partition halves
            # (row-tiled scores need lhsT/rhs at bases 0 and 64) ----
            qTd = jpool.tile([P, SQ], bf16, tag="qT")
            kTd = jpool.tile([P, SKV], bf16, tag="kT")

            def kproj(t):
                pp = ps_s.tile([P, 512], f32, tag="sp", name=f"kp{t}")
                for i in range(MB):
                    nc.tensor.matmul(
                        pp,
                        lhsT=wkk(i),
                        rhs=kcs[t][:, i, :],
                        start=(i == 0),
                        stop=(i == MB - 1),
                    )
                # psum f32 -> sbuf bf16 on ACT (keeps DVE for the mask)
                nc.scalar.copy(kTd[:, t * 512 : (t + 1) * 512], pp)

            def qproj(t):
                pp = ps_s.tile([P, 512], f32, tag="sp", name=f"qp{t}")
                for i in range(MB):
                    nc.tensor.matmul(
                        pp,
                        lhsT=wqq(i),
                        rhs=qcs[t][:, i, :],
                        start=(i == 0),
                        stop=(i == MB - 1),
                    )
                nc.scalar.copy(qTd[:, t * 512 : (t + 1) * 512], pp)

            kproj(0)
            qproj(0)
            qproj(1)

            # ---- per-j v-aug tiles (col 64 = ones) ----
            vaugs = []
            for j in range(JT):
                va = jpool.tile([P, D_K + 1], bf16, tag=f"va{j}", name=f"va{j}")
                nc.vector.memset(va[:, D_K : D_K + 1], 1.0)
                vaugs.append(va)

            # ---- per-pair E tiles [128, 2, 1024] ----
            Eps = [
                jpool.tile([P, 2, SQ], bf16, tag=f"E{p}", name=f"E{p}")
                for p in range(NPAIR)
            ]

            def E_of(j):
                return Eps[j // 2][:, j % 2, :]

            # ---- main pipeline over skv tile pairs.
            # Emission order is engine-queue order. Stagger: vproj one
            # pair late, PV two pairs late (interleaved with vproj so
            # the single pv psum buffer never stalls the PE).
            pvs = [None] * JT

            def emit_vproj(j):
                g, jj = divmod(j, JPG)
                pv = ps_pv.tile([P, D_K], f32, tag="pv", name=f"pv{j}")
                for i in range(MB):
                    nc.tensor.matmul(
                        pv,
                        lhsT=vgs[g][:, jj, i, :],
                        rhs=wv(i),
                        start=(i == 0),
                        stop=(i == MB - 1),
                    )
                pvs[j] = pv

            def emit_cast(j):
                nc.vector.tensor_copy(vaugs[j][:, 0:D_K], pvs[j])

            def emit_pv(j, chunks=(0, 1), finalize=False):
                for c in chunks:
                    csl = slice(c * 512, (c + 1) * 512)
                    nc.tensor.matmul(
                        oTp[:, csl],
                        lhsT=vaugs[j],
                        rhs=E_of(j)[:, csl],
                        start=(j == 0),
                        stop=(j == JT - 1),
                    )
                    if finalize:
                        nc.scalar.copy(oT_sb[:, csl], oTp[:, csl])
                        nc.sync.dma_start(out=out_d[:, csl], in_=oT_sb[:, csl])

            def emit_scores(p):
                # row-tiled: tile j0 at array rows 0-63, j1 at 64-127;
                # each (j, chunk) matmul gets its own psum bank, its
                # own exp, and its own [128, 512] mask pred
                j0 = 2 * p
                for c in range(NQC):
                    csl = slice(c * 512, (c + 1) * 512)
                    for h, j in ((0, j0), (1, j0 + 1)):
                        sp = ps_s.tile([P, 512], f32, tag="sp", name=f"s{j}c{c}")
                        nc.tensor.matmul(
                            sp,
                            lhsT=kTd[h * D_K : (h + 1) * D_K, j * P : (j + 1) * P],
                            rhs=qTd[h * D_K : (h + 1) * D_K, csl],
                            start=True,
                            stop=True,
                            tile_position=(h * D_K, 0),
                        )
                        # E = exp(s / sqrt(d_model)) straight from PSUM
                        nc.scalar.activation(
                            out=E_of(j)[:, csl],
                            in_=sp,
                            func=mybir.ActivationFunctionType.Exp,
                            scale=float(D_MODEL) ** -0.5,
                        )
                        # masked positions -> 1.0 (== exp(1e-9) in f32)
                        nc.vector.copy_predicated(
                            out=E_of(j)[:, csl],
                            mask=mps[p][:, j - j0, csl],
                            data=ones_bf,
                        )

            oT_sb = jpool.tile([D_K + 1, SQ], f32, tag="oTs")

            for p in range(NPAIR):
                emit_scores(p)
                if p % 2 == 0 and p // 2 + 1 < NKC:
                    kproj(p // 2 + 1)
                if p >= 1:
                    emit_vproj(2 * p - 2)
                    if p >= 2:
                        emit_pv(2 * p - 4)
                    emit_vproj(2 * p - 1)
                    emit_cast(2 * p - 2)
                    emit_cast(2 * p - 1)
                    if p >= 2:
                        emit_pv(2 * p - 3)

            # drain the staggered tail
            emit_vproj(JT - 2)
            emit_pv(JT - 4)
            emit_vproj(JT - 1)
            emit_cast(JT - 2)
            emit_cast(JT - 1)
            emit_pv(JT - 3)
            emit_pv(JT - 2)
            emit_pv(JT - 1, chunks=(0,), finalize=True)
            emit_pv(JT - 1, chunks=(1,), finalize=True)

    nc.finalize()
    return nc


def _get_nc():
    global _cached_nc
    if _cached_nc is None:
        _cached_nc = _build_nc()
    return _cached_nc


def _shard_inputs(query, key, value, mask, w_q, w_k, w_v):
    """Host-side shard + layout prep. Core c -> (batch c//2, q-half c%2)."""

    def pack_w(w, dt):
        # [d_k, m] -> [128, MB, d_k]
        return w.T.astype(dt).reshape(MB, P, D_K).transpose(1, 0, 2)

    # q/k weights duplicated along columns: [wq|wq|wk|wk]
    wq_p = pack_w(w_q, _FP8)
    wk_p = pack_w(w_k, _FP8)
    w_qk = np.ascontiguousarray(
        np.concatenate([wq_p, wq_p, wk_p, wk_p], axis=2)
    )
    w_v_p = np.ascontiguousarray(pack_w(w_v, _BF16))

    in_maps = []
    for c in range(N_CORES):
        b, h = divmod(c, 2)
        s0 = h * SQ
        q_t = query[b, s0 : s0 + SQ, :].T.astype(_FP8)     # [m, sq]
        k_t = key[b].T.astype(_FP8)                        # [m, skv]
        v_t = value[b].T.astype(_BF16)                     # [m, skv]
        m_t = mask[b, s0 : s0 + SQ, :].T.astype(np.uint8)  # [skv, sq]
        in_maps.append(
            {
                "w_qk": w_qk,
                "w_v": w_v_p,
                # [m, sq] -> [128, NQC, MB, 512]
                "q_t": np.ascontiguousarray(
                    q_t.reshape(MB, P, NQC, 512).transpose(1, 2, 0, 3)
                ),
                # [m, skv] -> [128, NKC, MB, 512]
                "k_t": np.ascontiguousarray(
                    k_t.reshape(MB, P, NKC, 512).transpose(1, 2, 0, 3)
                ),
                # [m, skv] -> [128, VG, JPG, MB, 128]
                "v_t": np.ascontiguousarray(
                    v_t.reshape(MB, P, VG, JPG, P).transpose(1, 2, 3, 0, 4)
                ),
                # [skv, sq] -> [128, NPAIR, 2, SQ]
                "mask_t": np.ascontiguousarray(
                    m_t.reshape(NPAIR, 2, P, SQ).transpose(2, 0, 1, 3)
                ),
            }
        )
    return in_maps


def run(inputs, trace=False):
    """Run the SPMD kernel; returns (output [B,S,D_K] f32, BassKernelResults)."""
    from concourse.bass_utils import run_bass_kernel_spmd

    nc = _get_nc()
    in_maps = _shard_inputs(**inputs)
    res = run_bass_kernel_spmd(
        nc, in_maps, core_ids=list(range(N_CORES)), trace=trace
    )
    out = np.empty((B, S, D_K), np.float32)
    for c in range(N_CORES):
        b, h = divmod(c, 2)
        # device out is oT [65, SQ]: rows 0-63 = numerator.T, row 64 =
        # softmax denominator
        oT = res.results[c]["out"]
        out[b, h * SQ : (h + 1) * SQ, :] = (oT[0:D_K] / oT[D_K : D_K + 1]).T
    return out, res


def kernel(**inputs):
    out, _ = run(inputs, trace=False)
    return out


# revision 9
# speedup vs baseline: 1.2595x; 1.0076x over previous
"""AttentionHead kernel for 8 Trainium2 NeuronCores (SPMD data-parallel).

Problem: q/k/v projections [1024->64] + masked softmax attention,
B=4, S=2048, d_model=1024, d_k=64.

Sharding: 8 cores = 4 batches x 2 kv-halves. Each core handles one
(batch, kv-half): full query [2048, 1024], key/value half [1024, 1024],
mask shard [kv 1024, q 2048]. Weights replicated. Each core produces a
partial attention numerator + denominator over its kv half; the host
adds the two halves and divides (flash-attention style combine; exps
need no max-shift since |scores| < ~2.5).

Precision: raw query/key and w_q/w_k ship as fp8e4m3 (halves their DMA
bytes; scores only need ~2 decimal digits since they sit in an exp).
value/w_v stay bf16 (the output is linear in v).

Per-core device pipeline (everything contracts on the partition dim;
all inputs host-packed so each DMA is one contiguous transfer, ordered
just-in-time; the q dimension is processed in two 1024-col phases so
the [65, 1024] f32 PV accumulator fits in 2 PSUM banks):
  - warm-up matmuls on a memset junk tile (no DMA dependency) ramp the
    PE clock until k0 lands; they accumulate garbage into the PV psum
    region, which PV's start=True later clears
  - q/k projection weights are host-duplicated [w|w] so one matmul per
    m-block yields the projection replicated across both partition
    halves (128-col stationary -> fast weight load)
  - scores are computed TRANSPOSED [skv_tile=128, sq] and row-tiled:
    tiles j (array rows 0-63, kT/qT partitions 0-63) and j+1 (rows
    64-127, the duplicated halves) run concurrently since the
    contraction (d_k) is only 64; both 512-col chunks of a tile land
    in one [128, 1024] 2-bank psum tile -> a single merged exp
  - ACT exp reads scores straight from PSUM with the 1024**-0.5 scale
    fused; masked positions then set to 1.0 (==exp(1e-9) in fp32) via
    per-(tile, chunk) copy_predicated [128, 512] on the bf16 E tiles
  - PV accumulates transposed: oT[65, q] += vaug_j.T @ E_j; row 64 is
    the partial softmax denominator (free via the ones column of vaug)
  - finalize per phase: per-chunk psum->sbuf copy + output DMA of
    oT [65, 1024]; transpose/combine/divide run on the host
"""

import numpy as np
import ml_dtypes

B = 4
S = 2048
D_MODEL = 1024
D_K = 64
N_CORES = 8

P = 128
SQF = S              # per-core query rows (full 2048)
SKV = S // 2         # per-core kv rows (1024)
SQH = 1024           # q columns per phase
NPH = 2              # q phases
MB = D_MODEL // P    # 8 m-blocks (contraction)
JT = SKV // P        # 8 skv tiles
NQC = SQF // 512     # 4 q chunks
NKC = SKV // 512     # 2 k chunks
NPAIR = JT // 2      # 4 skv tile pairs
VG = 4               # v DMA groups (2 skv tiles each)
JPG = JT // VG       # skv tiles per v group (2)

_BF16 = ml_dtypes.bfloat16
_FP8 = ml_dtypes.float8_e4m3

_cached_nc = None


def _build_nc():
    import concourse.mybir as mybir
    import concourse.tile as tile
    from concourse import bacc

    bf16 = mybir.dt.bfloat16
    fp8 = mybir.dt.float8e4
    f32 = mybir.dt.float32
    u8 = mybir.dt.uint8

    nc = bacc.Bacc(None, target_bir_lowering=False)

    wqk_d = nc.dram_tensor("w_qk", [P, MB, 4 * D_K], fp8, kind="ExternalInput")
    wv_d = nc.dram_tensor("w_v", [P, MB, D_K], bf16, kind="ExternalInput")
    q_d = nc.dram_tensor("q_t", [P, NQC, MB, 512], fp8, kind="ExternalInput")
    k_d = nc.dram_tensor("k_t", [P, NKC, MB, 512], fp8, kind="ExternalInput")
    v_d = nc.dram_tensor("v_t", [P, VG, JPG, MB, P], bf16, kind="ExternalInput")
    m_d = nc.dram_tensor(
        "mask_t", [P, NPH, NPAIR, 2, SQH], u8, kind="ExternalInput"
    )
    out_d = nc.dram_tensor("out", [D_K + 1, SQF], f32, kind="ExternalOutput")

    with tile.TileContext(nc) as tc:
        with (
            tc.tile_pool(name="const", bufs=1) as cpool,
            tc.tile_pool(name="inp", bufs=1) as ipool,
            tc.tile_pool(name="proj", bufs=1) as jpool,
            tc.tile_pool(name="ps_s", bufs=4, space="PSUM") as ps_s,
            tc.tile_pool(name="ps_pv", bufs=2, space="PSUM") as ps_pv,
            tc.tile_pool(name="ps_o", bufs=1, space="PSUM") as ps_o,
        ):
            # ---- junk tile for DMA-independent PE warm-up ----
            junk = cpool.tile([P, 512], bf16, tag="junk")
            nc.vector.memset(junk, 0.125)

            # ---- input DMAs: emission order == stream order (HWDGE
            # FIFO); ordered so each tensor lands just before its
            # consumer is ready. The first k/q chunks are split into
            # m-block halves so projections start mid-transfer. ----
            wqk_sb = cpool.tile([P, MB, 4 * D_K], fp8, tag="wqk")
            nc.sync.dma_start(out=wqk_sb, in_=wqk_d[:])
            wv_sb = cpool.tile([P, MB, D_K], bf16, tag="wv")

            qcs = [None] * NQC
            kcs = [None] * NKC
            mps = [[None] * NPAIR for _ in range(NPH)]
            vgs = [None] * VG

            def dma_q(t, split=False):
                qc = ipool.tile([P, MB, 512], fp8, tag=f"q{t}", name=f"q{t}")
                if split:
                    nc.sync.dma_start(out=qc[:, 0:4], in_=q_d[:, t, 0:4])
                    nc.sync.dma_start(out=qc[:, 4:8], in_=q_d[:, t, 4:8])
                else:
                    nc.sync.dma_start(out=qc, in_=q_d[:, t])
                qcs[t] = qc

            def dma_k(t, split=False):
                kc = ipool.tile([P, MB, 512], fp8, tag=f"k{t}", name=f"k{t}")
                if split:
                    nc.sync.dma_start(out=kc[:, 0:4], in_=k_d[:, t, 0:4])
                    nc.sync.dma_start(out=kc[:, 4:8], in_=k_d[:, t, 4:8])
                else:
                    nc.sync.dma_start(out=kc, in_=k_d[:, t])
                kcs[t] = kc

            def dma_m(ph, p):
                mp = ipool.tile([P, 2, SQH], u8, tag=f"m{ph}{p}", name=f"m{ph}{p}")
                nc.sync.dma_start(out=mp, in_=m_d[:, ph, p])
                mps[ph][p] = mp

            def dma_v(g):
                vg = ipool.tile([P, JPG, MB, P], bf16, tag=f"v{g}", name=f"v{g}")
                nc.sync.dma_start(out=vg, in_=v_d[:, g])
                vgs[g] = vg

            dma_k(0, split=True)
            dma_q(0, split=True)
            dma_m(0, 0)
            dma_q(1)
            dma_k(1)
            nc.sync.dma_start(out=wv_sb, in_=wv_d[:])
            dma_v(0)
            dma_m(0, 1)
            dma_v(1)
            dma_q(2)
            dma_m(0, 2)
            dma_v(2)
            dma_q(3)
            dma_m(0, 3)
            dma_v(3)
            dma_m(1, 0)
            dma_m(1, 1)
            dma_m(1, 2)
            dma_m(1, 3)

            def wqq(i):  # [wq | wq], 128 cols
                return wqk_sb[:, i, 0 : 2 * D_K]

            def wkk(i):  # [wk | wk], 128 cols
                return wqk_sb[:, i, 2 * D_K : 4 * D_K]

            def wv(i):
                return wv_sb[:, i, :]

            # ---- constants ----
            ones_bf = cpool.tile([P, 512], bf16, tag="ones")
            nc.vector.memset(ones_bf, 1.0)

            # ---- phase PV accumulator [65, 1024] f32 = 2 psum banks,
            # reused across the two q phases ----
            oTp = ps_o.tile([D_K + 1, SQH], f32, tag="oT")

            # ---- PE warm-up: junk matmuls bridge the gap until k0
            # lands, ramping the HAM clock; they accumulate into oTp,
            # which PV's start=True later clears ----
            for wi in range(13):
                nc.tensor.matmul(
                    oTp[0:D_K, 0:512],
                    lhsT=junk[:, 0:D_K],
                    rhs=junk,
                    start=(wi == 0),
                    stop=(wi == 12),
                )

            # ---- projections: host-duplicated weights yield the
            # projected tensor replicated across both partition halves
            # (row-tiled scores need lhsT/rhs at bases 0 and 64).
            # Projections borrow a [P, 1024] scores psum tile (half
            # used) so psum stays within 8 banks. ----
            qTd = jpool.tile([P, SQF], bf16, tag="qT")
            kTd = jpool.tile([P, SKV], bf16, tag="kT")

            def proj(w_of, src, dst, name):
                pp = ps_s.tile([P, 512], f32, tag="sp", name=name)
                for i in range(MB):
                    nc.tensor.matmul(
                        pp,
                        lhsT=w_of(i),
                        rhs=src[:, i, :],
                        start=(i == 0),
                        stop=(i == MB - 1),
                    )
                # psum f32 -> sbuf bf16 on ACT (keeps DVE for the mask)
                nc.scalar.copy(dst, pp)

            def kproj(t):
                proj(wkk, kcs[t], kTd[:, t * 512 : (t + 1) * 512], f"kp{t}")

            def qproj(t):
                proj(wqq, qcs[t], qTd[:, t * 512 : (t + 1) * 512], f"qp{t}")

            # ---- per-j v-aug tiles (col 64 = ones) ----
            vaugs = []
            for j in range(JT):
                va = jpool.tile([P, D_K + 1], bf16, tag=f"va{j}", name=f"va{j}")
                nc.vector.memset(va[:, D_K : D_K + 1], 1.0)
                vaugs.append(va)

            # ---- per-pair E tiles [128, 2, 1024], reused across the
            # two phases ----
            Eps = [
                jpool.tile([P, 2, SQH], bf16, tag=f"E{p}", name=f"E{p}")
                for p in range(NPAIR)
            ]

            def E_of(j):
                return Eps[j // 2][:, j % 2, :]

            oT_sb = jpool.tile([D_K + 1, SQF], f32, tag="oTs")

            pvs = [None] * JT

            def emit_vproj(j):
                g, jj = divmod(j, JPG)
                pv = ps_pv.tile([P, D_K], f32, tag="pv", name=f"pv{j}")
                for i in range(MB):
                    nc.tensor.matmul(
                        pv,
                        lhsT=vgs[g][:, jj, i, :],
                        rhs=wv(i),
                        start=(i == 0),
                        stop=(i == MB - 1),
                    )
                pvs[j] = pv
                nc.vector.tensor_copy(vaugs[j][:, 0:D_K], pv)

            def emit_pv(ph, j, finalize=False):
                for c in range(2):
                    csl = slice(c * 512, (c + 1) * 512)
                    nc.tensor.matmul(
                        oTp[:, csl],
                        lhsT=vaugs[j],
                        rhs=E_of(j)[:, csl],
                        start=(j == 0),
                        stop=(j == JT - 1),
                    )
                    if finalize:
                        osl = slice(ph * SQH + c * 512, ph * SQH + (c + 1) * 512)
                        nc.scalar.copy(oT_sb[:, osl], oTp[:, csl])
                        nc.sync.dma_start(out=out_d[:, osl], in_=oT_sb[:, osl])

            def emit_score_pair(ph, p, between=None):
                """One kv tile pair's scores for this phase: row-tiled
                (tile h=0 at array rows 0-63, h=1 at rows 64-127, run
                concurrently), one [128, 512] psum tile + exp + mask
                pred per (tile, chunk). `between` emits filler PE work
                between the two chunk rounds."""
                for c in range(2):
                    csl = slice(c * 512, (c + 1) * 512)
                    qsl = slice(ph * SQH + c * 512, ph * SQH + (c + 1) * 512)
                    for h in range(2):
                        j = 2 * p + h
                        sp = ps_s.tile(
                            [P, 512], f32, tag="sp", name=f"s{ph}_{j}c{c}"
                        )
                        nc.tensor.matmul(
                            sp,
                            lhsT=kTd[h * D_K : (h + 1) * D_K, j * P : (j + 1) * P],
                            rhs=qTd[h * D_K : (h + 1) * D_K, qsl],
                            start=True,
                            stop=True,
                            tile_position=(h * D_K, 0),
                        )
                        # E = exp(s / sqrt(d_model)) straight from PSUM
                        nc.scalar.activation(
                            out=E_of(j)[:, csl],
                            in_=sp,
                            func=mybir.ActivationFunctionType.Exp,
                            scale=float(D_MODEL) ** -0.5,
                        )
                        # masked positions -> 1.0 (== exp(1e-9) in f32)
                        nc.vector.copy_predicated(
                            out=E_of(j)[:, csl],
                            mask=mps[ph][p][:, h, csl],
                            data=ones_bf,
                        )
                    if c == 0 and between is not None:
                        between()

            # ---- prologue: kp0 / qp0 as soon as their data lands;
            # qp1 squeezed between the first two score matmuls ----
            kproj(0)
            qproj(0)

            # ---- the two q phases; "filler" PE work (projections,
            # vproj, PV of earlier pairs) rides between/after score
            # tiles so the PE queue never waits on a long dep chain ----
            fillers = {
                (0, 0): [lambda: qproj(1)],
                (0, 1): [lambda: kproj(1), lambda: emit_vproj(0),
                         lambda: emit_vproj(1), lambda: qproj(2)],
                (0, 2): [lambda: emit_vproj(2), lambda: emit_vproj(3),
                         lambda: emit_pv(0, 0), lambda: emit_pv(0, 1)],
                (0, 3): [lambda: emit_vproj(4), lambda: emit_vproj(5),
                         lambda: qproj(3), lambda: emit_pv(0, 2),
                         lambda: emit_pv(0, 3)],
                (1, 0): [lambda: emit_vproj(6), lambda: emit_vproj(7),
                         lambda: emit_pv(0, 4), lambda: emit_pv(0, 5)],
                (1, 1): [lambda: emit_pv(0, 6), lambda: emit_pv(0, 7, True)],
                (1, 2): [lambda: emit_pv(1, 0), lambda: emit_pv(1, 1),
                         lambda: emit_pv(1, 2)],
                (1, 3): [lambda: emit_pv(1, 3), lambda: emit_pv(1, 4)],
            }
            for ph in range(NPH):
                for p in range(NPAIR):
                    work = list(fillers.get((ph, p), []))
                    # first filler rides between the two chunk rounds
                    emit_score_pair(
                        ph, p, between=(work[0] if work else None)
                    )
                    for f in work[1:]:
                        f()
            emit_pv(1, 5)
            emit_pv(1, 6)
            emit_pv(1, 7, True)

    nc.finalize()
    return nc


def _get_nc():
    global _cached_nc
    if _cached_nc is None:
        _cached_nc = _build_nc()
    return _cached_nc


def _shard_inputs(query, key, value, mask, w_q, w_k, w_v):
    """Host-side shard + layout prep. Core c -> (batch c//2, kv-half c%2)."""

    def pack_w(w, dt):
        # [d_k, m] -> [128, MB, d_k]
        return w.T.astype(dt).reshape(MB, P, D_K).transpose(1, 0, 2)

    wq_p = pack_w(w_q, _FP8)
    wk_p = pack_w(w_k, _FP8)
    w_qk = np.ascontiguousarray(np.concatenate([wq_p, wq_p, wk_p, wk_p], axis=2))
    w_v_p = np.ascontiguousarray(pack_w(w_v, _BF16))

    in_maps = []
    for c in range(N_CORES):
        b, h = divmod(c, 2)
        s0 = h * SKV
        q_t = query[b].T.astype(_FP8)                       # [m, 2048]
        k_t = key[b, s0 : s0 + SKV, :].T.astype(_FP8)       # [m, 1024]
        v_t = value[b, s0 : s0 + SKV, :].T.astype(_BF16)    # [m, 1024]
        m_t = mask[b, :, s0 : s0 + SKV].T.astype(np.uint8)  # [kv 1024, q 2048]
        in_maps.append(
            {
                "w_qk": w_qk,
                "w_v": w_v_p,
                # [m, 2048] -> [128, NQC, MB, 512]
                "q_t": np.ascontiguousarray(
                    q_t.reshape(MB, P, NQC, 512).transpose(1, 2, 0, 3)
                ),
                # [m, 1024] -> [128, NKC, MB, 512]
                "k_t": np.ascontiguousarray(
                    k_t.reshape(MB, P, NKC, 512).transpose(1, 2, 0, 3)
                ),
                # [m, 1024] -> [128, VG, JPG, MB, 128]
                "v_t": np.ascontiguousarray(
                    v_t.reshape(MB, P, VG, JPG, P).transpose(1, 2, 3, 0, 4)
                ),
                # [kv 1024, q 2048] -> [128, NPH, NPAIR, 2, SQH]
                "mask_t": np.ascontiguousarray(
                    m_t.reshape(NPAIR, 2, P, NPH, SQH).transpose(2, 3, 0, 1, 4)
                ),
            }
        )
    return in_maps


def run(inputs, trace=False):
    """Run the SPMD kernel; returns (output [B,S,D_K] f32, BassKernelResults)."""
    from concourse.bass_utils import run_bass_kernel_spmd

    nc = _get_nc()
    in_maps = _shard_inputs(**inputs)
    res = run_bass_kernel_spmd(
        nc, in_maps, core_ids=list(range(N_CORES)), trace=trace
    )
    out = np.empty((B, S, D_K), np.float32)
    for b in range(B):
        # each kv-half core returns partial oT [65, 2048]: rows 0-63 =
        # numerator.T, row 64 = partial softmax denominator
        oT = res.results[2 * b]["out"] + res.results[2 * b + 1]["out"]
        out[b] = (oT[0:D_K] / oT[D_K : D_K + 1]).T
    return out, res


def kernel(**inputs):
    out, _ = run(inputs, trace=False)
    return out


# revision 11
# speedup vs baseline: 1.3148x; 1.0439x over previous
"""AttentionHead kernel for 8 Trainium2 NeuronCores (SPMD data-parallel).

Problem: q/k/v projections [1024->64] + masked softmax attention,
B=4, S=2048, d_model=1024, d_k=64.

Sharding: 8 cores = 4 batches x 2 kv-halves. Each core handles one
(batch, kv-half): full query [2048, 1024], key/value half [1024, 1024],
mask shard [kv 1024, q 2048]. Weights replicated. Each core produces a
partial attention numerator + denominator over its kv half; the host
adds the two halves and divides (flash-attention style combine; exps
need no max-shift since |scores| < ~2.5).

Precision: raw query/key and w_q/w_k ship as fp8e4m3 (halves their DMA
bytes; scores only need ~2 decimal digits since they sit in an exp).
value/w_v stay bf16 (the output is linear in v).

Per-core device pipeline (everything contracts on the partition dim;
all inputs host-packed so each DMA is one contiguous transfer, ordered
just-in-time; the q dimension is processed in two 1024-col phases so
the [65, 1024] f32 PV accumulator fits in 2 PSUM banks):
  - warm-up matmuls on a memset junk tile (no DMA dependency) ramp the
    PE clock until k0 lands; they accumulate garbage into the PV psum
    region, which PV's start=True later clears
  - q/k projection weights are host-duplicated [w|w] so one matmul per
    m-block yields the projection replicated across both partition
    halves (128-col stationary -> fast weight load)
  - scores are computed TRANSPOSED [skv_tile=128, sq] and row-tiled:
    tiles j (array rows 0-63, kT/qT partitions 0-63) and j+1 (rows
    64-127, the duplicated halves) run concurrently since the
    contraction (d_k) is only 64; both 512-col chunks of a tile land
    in one [128, 1024] 2-bank psum tile -> a single merged exp
  - ACT exp reads scores straight from PSUM with the 1024**-0.5 scale
    fused; masked positions then set to 1.0 (==exp(1e-9) in fp32) via
    per-(tile, chunk) copy_predicated [128, 512] on the bf16 E tiles
  - PV accumulates transposed: oT[65, q] += vaug_j.T @ E_j; row 64 is
    the partial softmax denominator (free via the ones column of vaug)
  - finalize per phase: per-chunk psum->sbuf copy + output DMA of
    oT [65, 1024]; transpose/combine/divide run on the host
"""

import numpy as np
import ml_dtypes

B = 4
S = 2048
D_MODEL = 1024
D_K = 64
N_CORES = 8

P = 128
SQF = S              # per-core query rows (full 2048)
SKV = S // 2         # per-core kv rows (1024)
SQH = 1024           # q columns per phase
NPH = 2              # q phases
MB = D_MODEL // P    # 8 m-blocks (contraction)
JT = SKV // P        # 8 skv tiles
NQC = SQF // 512     # 4 q chunks
NKC = SKV // 512     # 2 k chunks
NPAIR = JT // 2      # 4 skv tile pairs
VG = 4               # v DMA groups (2 skv tiles each)
JPG = JT // VG       # skv tiles per v group (2)

_BF16 = ml_dtypes.bfloat16
_FP8 = ml_dtypes.float8_e4m3

_cached_nc = None


def _build_nc():
    import concourse.mybir as mybir
    import concourse.tile as tile
    from concourse import bacc

    bf16 = mybir.dt.bfloat16
    fp8 = mybir.dt.float8e4
    f32 = mybir.dt.float32
    u8 = mybir.dt.uint8

    nc = bacc.Bacc(None, target_bir_lowering=False)

    wqk_d = nc.dram_tensor("w_qk", [P, MB, 4 * D_K], fp8, kind="ExternalInput")
    wv_d = nc.dram_tensor("w_v", [P, MB, D_K], bf16, kind="ExternalInput")
    q_d = nc.dram_tensor("q_t", [P, NQC, MB, 512], fp8, kind="ExternalInput")
    k_d = nc.dram_tensor("k_t", [P, NKC, MB, 512], fp8, kind="ExternalInput")
    v_d = nc.dram_tensor("v_t", [P, VG, JPG, MB, P], bf16, kind="ExternalInput")
    m_d = nc.dram_tensor(
        "mask_t", [P, NPH, NPAIR, 2, SQH], u8, kind="ExternalInput"
    )
    out_d = nc.dram_tensor("out", [D_K + 1, SQF], f32, kind="ExternalOutput")

    with tile.TileContext(nc) as tc:
        with (
            tc.tile_pool(name="const", bufs=1) as cpool,
            tc.tile_pool(name="inp", bufs=1) as ipool,
            tc.tile_pool(name="proj", bufs=1) as jpool,
            tc.tile_pool(name="ps_s", bufs=4, space="PSUM") as ps_s,
            tc.tile_pool(name="ps_pv", bufs=2, space="PSUM") as ps_pv,
            tc.tile_pool(name="ps_o", bufs=1, space="PSUM") as ps_o,
        ):
            # ---- junk tile for DMA-independent PE warm-up ----
            junk = cpool.tile([P, 512], bf16, tag="junk")
            nc.vector.memset(junk, 0.125)

            # ---- input DMAs: emission order == stream order (HWDGE
            # FIFO); ordered so each tensor lands just before its
            # consumer is ready. The first k/q chunks are split into
            # m-block halves so projections start mid-transfer. ----
            wqk_sb = cpool.tile([P, MB, 4 * D_K], fp8, tag="wqk")
            nc.sync.dma_start(out=wqk_sb, in_=wqk_d[:])
            wv_sb = cpool.tile([P, MB, D_K], bf16, tag="wv")

            qcs = [None] * NQC
            kcs = [None] * NKC
            mps = [[None] * NPAIR for _ in range(NPH)]
            vgs = [None] * VG

            def dma_q(t, split=False):
                qc = ipool.tile([P, MB, 512], fp8, tag=f"q{t}", name=f"q{t}")
                if split:
                    nc.sync.dma_start(out=qc[:, 0:4], in_=q_d[:, t, 0:4])
                    nc.sync.dma_start(out=qc[:, 4:8], in_=q_d[:, t, 4:8])
                else:
                    nc.sync.dma_start(out=qc, in_=q_d[:, t])
                qcs[t] = qc

            def dma_k(t, split=False):
                kc = ipool.tile([P, MB, 512], fp8, tag=f"k{t}", name=f"k{t}")
                if split:
                    nc.sync.dma_start(out=kc[:, 0:4], in_=k_d[:, t, 0:4])
                    nc.sync.dma_start(out=kc[:, 4:8], in_=k_d[:, t, 4:8])
                else:
                    nc.sync.dma_start(out=kc, in_=k_d[:, t])
                kcs[t] = kc

            def dma_m(ph, p):
                mp = ipool.tile([P, 2, SQH], u8, tag=f"m{ph}{p}", name=f"m{ph}{p}")
                nc.sync.dma_start(out=mp, in_=m_d[:, ph, p])
                mps[ph][p] = mp

            def dma_v(g):
                vg = ipool.tile([P, JPG, MB, P], bf16, tag=f"v{g}", name=f"v{g}")
                nc.sync.dma_start(out=vg, in_=v_d[:, g])
                vgs[g] = vg

            dma_k(0, split=True)
            dma_q(0, split=True)
            dma_m(0, 0)
            dma_q(1)
            dma_k(1)
            nc.sync.dma_start(out=wv_sb, in_=wv_d[:])
            dma_v(0)
            dma_m(0, 1)
            dma_v(1)
            dma_q(2)
            dma_m(0, 2)
            dma_v(2)
            dma_q(3)
            dma_m(0, 3)
            dma_v(3)
            dma_m(1, 0)
            dma_m(1, 1)
            dma_m(1, 2)
            dma_m(1, 3)

            def wqq(i):  # [wq | wq] for m-blocks i, i+1: [128, 2, 128]
                return wqk_sb[:, i : i + 2, 0 : 2 * D_K]

            def wkk(i):  # [wk | wk] for m-blocks i, i+1: [128, 2, 128]
                return wqk_sb[:, i : i + 2, 2 * D_K : 4 * D_K]

            def wv(i):
                return wv_sb[:, i, :]

            # ---- constants ----
            ones_bf = cpool.tile([P, 512], bf16, tag="ones")
            nc.vector.memset(ones_bf, 1.0)

            # ---- phase PV accumulator [65, 1024] f32 = 2 psum banks,
            # reused across the two q phases ----
            oTp = ps_o.tile([D_K + 1, SQH], f32, tag="oT")

            # ---- PE warm-up: junk matmuls bridge the gap until k0
            # lands, ramping the HAM clock; they accumulate into oTp,
            # which PV's start=True later clears ----
            for wi in range(13):
                nc.tensor.matmul(
                    oTp[0:D_K, 0:512],
                    lhsT=junk[:, 0:D_K],
                    rhs=junk,
                    start=(wi == 0),
                    stop=(wi == 12),
                )

            # ---- projections: host-duplicated weights yield the
            # projected tensor replicated across both partition halves
            # (row-tiled scores need lhsT/rhs at bases 0 and 64).
            # Projections borrow a [P, 1024] scores psum tile (half
            # used) so psum stays within 8 banks. ----
            qTd = jpool.tile([P, SQF], bf16, tag="qT")
            kTd = jpool.tile([P, SKV], bf16, tag="kT")

            def proj(w_of, src, dst, name):
                # fp8 DoubleRow: two m-blocks (256-deep contraction) per
                # matmul; lhsT/rhs dim 1 indexes the two reduction tiles
                pp = ps_s.tile([P, 512], f32, tag="sp", name=name)
                for i in range(0, MB, 2):
                    nc.tensor.matmul(
                        pp,
                        lhsT=w_of(i),
                        rhs=src[:, i : i + 2, :],
                        start=(i == 0),
                        stop=(i == MB - 2),
                        perf_mode=mybir.MatmulPerfMode.DoubleRow,
                    )
                # psum f32 -> sbuf bf16 on ACT (keeps DVE for the mask)
                nc.scalar.copy(dst, pp)

            def kproj(t):
                proj(wkk, kcs[t], kTd[:, t * 512 : (t + 1) * 512], f"kp{t}")

            def qproj(t):
                proj(wqq, qcs[t], qTd[:, t * 512 : (t + 1) * 512], f"qp{t}")

            # ---- per-j v-aug tiles (col 64 = ones) ----
            vaugs = []
            for j in range(JT):
                va = jpool.tile([P, D_K + 1], bf16, tag=f"va{j}", name=f"va{j}")
                nc.vector.memset(va[:, D_K : D_K + 1], 1.0)
                vaugs.append(va)

            # ---- per-pair E tiles [128, 2, 1024], reused across the
            # two phases ----
            Eps = [
                jpool.tile([P, 2, SQH], bf16, tag=f"E{p}", name=f"E{p}")
                for p in range(NPAIR)
            ]

            def E_of(j):
                return Eps[j // 2][:, j % 2, :]

            oT_sb = jpool.tile([D_K + 1, SQF], f32, tag="oTs")

            pvs = [None] * JT

            def emit_vproj(j):
                g, jj = divmod(j, JPG)
                pv = ps_pv.tile([P, D_K], f32, tag="pv", name=f"pv{j}")
                for i in range(MB):
                    nc.tensor.matmul(
                        pv,
                        lhsT=vgs[g][:, jj, i, :],
                        rhs=wv(i),
                        start=(i == 0),
                        stop=(i == MB - 1),
                    )
                pvs[j] = pv
                nc.vector.tensor_copy(vaugs[j][:, 0:D_K], pv)

            def emit_pv(ph, j, finalize=False):
                for c in range(2):
                    csl = slice(c * 512, (c + 1) * 512)
                    nc.tensor.matmul(
                        oTp[:, csl],
                        lhsT=vaugs[j],
                        rhs=E_of(j)[:, csl],
                        start=(j == 0),
                        stop=(j == JT - 1),
                    )
                    if finalize:
                        osl = slice(ph * SQH + c * 512, ph * SQH + (c + 1) * 512)
                        nc.scalar.copy(oT_sb[:, osl], oTp[:, csl])
                        nc.sync.dma_start(out=out_d[:, osl], in_=oT_sb[:, osl])

            def emit_score_pair(ph, p, between=None):
                """One kv tile pair's scores for this phase: row-tiled
                (tile h=0 at array rows 0-63, h=1 at rows 64-127, run
                concurrently), one [128, 512] psum tile + exp + mask
                pred per (tile, chunk). `between` emits filler PE work
                between the two chunk rounds."""
                for c in range(2):
                    csl = slice(c * 512, (c + 1) * 512)
                    qsl = slice(ph * SQH + c * 512, ph * SQH + (c + 1) * 512)
                    for h in range(2):
                        j = 2 * p + h
                        sp = ps_s.tile(
                            [P, 512], f32, tag="sp", name=f"s{ph}_{j}c{c}"
                        )
                        nc.tensor.matmul(
                            sp,
                            lhsT=kTd[h * D_K : (h + 1) * D_K, j * P : (j + 1) * P],
                            rhs=qTd[h * D_K : (h + 1) * D_K, qsl],
                            start=True,
                            stop=True,
                            tile_position=(h * D_K, 0),
                        )
                        # E = exp(s / sqrt(d_model)) straight from PSUM
                        nc.scalar.activation(
                            out=E_of(j)[:, csl],
                            in_=sp,
                            func=mybir.ActivationFunctionType.Exp,
                            scale=float(D_MODEL) ** -0.5,
                        )
                        # masked positions -> 1.0 (== exp(1e-9) in f32)
                        nc.vector.copy_predicated(
                            out=E_of(j)[:, csl],
                            mask=mps[ph][p][:, h, csl],
                            data=ones_bf,
                        )
                    if c == 0 and between is not None:
                        between()

            # ---- prologue: kp0 / qp0 as soon as their data lands;
            # qp1 squeezed between the first two score matmuls ----
            kproj(0)
            qproj(0)

            # ---- the two q phases; "filler" PE work (projections,
            # vproj, PV of earlier pairs) rides between/after score
            # tiles so the PE queue never waits on a long dep chain ----
            fillers = {
                (0, 0): [lambda: qproj(1)],
                (0, 1): [lambda: kproj(1), lambda: emit_vproj(0),
                         lambda: emit_vproj(1), lambda: qproj(2)],
                (0, 2): [lambda: emit_vproj(2), lambda: emit_vproj(3),
                         lambda: emit_pv(0, 0), lambda: emit_pv(0, 1)],
                (0, 3): [lambda: emit_vproj(4), lambda: emit_vproj(5),
                         lambda: qproj(3), lambda: emit_pv(0, 2),
                         lambda: emit_pv(0, 3)],
                (1, 0): [lambda: emit_vproj(6), lambda: emit_vproj(7),
                         lambda: emit_pv(0, 4), lambda: emit_pv(0, 5)],
                (1, 1): [lambda: emit_pv(0, 6), lambda: emit_pv(0, 7, True)],
                (1, 2): [lambda: emit_pv(1, 0), lambda: emit_pv(1, 1),
                         lambda: emit_pv(1, 2)],
                (1, 3): [lambda: emit_pv(1, 3), lambda: emit_pv(1, 4)],
            }
            for ph in range(NPH):
                for p in range(NPAIR):
                    work = list(fillers.get((ph, p), []))
                    # first filler rides between the two chunk rounds
                    emit_score_pair(
                        ph, p, between=(work[0] if work else None)
                    )
                    for f in work[1:]:
                        f()
            emit_pv(1, 5)
            emit_pv(1, 6)
            emit_pv(1, 7, True)

    nc.finalize()
    return nc


def _get_nc():
    global _cached_nc
    if _cached_nc is None:
        _cached_nc = _build_nc()
    return _cached_nc


def _shard_inputs(query, key, value, mask, w_q, w_k, w_v):
    """Host-side shard + layout prep. Core c -> (batch c//2, kv-half c%2)."""

    def pack_w(w, dt):
        # [d_k, m] -> [128, MB, d_k]
        return w.T.astype(dt).reshape(MB, P, D_K).transpose(1, 0, 2)

    wq_p = pack_w(w_q, _FP8)
    wk_p = pack_w(w_k, _FP8)
    w_qk = np.ascontiguousarray(np.concatenate([wq_p, wq_p, wk_p, wk_p], axis=2))
    w_v_p = np.ascontiguousarray(pack_w(w_v, _BF16))

    in_maps = []
    for c in range(N_CORES):
        b, h = divmod(c, 2)
        s0 = h * SKV
        q_t = query[b].T.astype(_FP8)                       # [m, 2048]
        k_t = key[b, s0 : s0 + SKV, :].T.astype(_FP8)       # [m, 1024]
        v_t = value[b, s0 : s0 + SKV, :].T.astype(_BF16)    # [m, 1024]
        m_t = mask[b, :, s0 : s0 + SKV].T.astype(np.uint8)  # [kv 1024, q 2048]
        in_maps.append(
            {
                "w_qk": w_qk,
                "w_v": w_v_p,
                # [m, 2048] -> [128, NQC, MB, 512]
                "q_t": np.ascontiguousarray(
                    q_t.reshape(MB, P, NQC, 512).transpose(1, 2, 0, 3)
                ),
                # [m, 1024] -> [128, NKC, MB, 512]
                "k_t": np.ascontiguousarray(
                    k_t.reshape(MB, P, NKC, 512).transpose(1, 2, 0, 3)
                ),
                # [m, 1024] -> [128, VG, JPG, MB, 128]
                "v_t": np.ascontiguousarray(
                    v_t.reshape(MB, P, VG, JPG, P).transpose(1, 2, 3, 0, 4)
                ),
                # [kv 1024, q 2048] -> [128, NPH, NPAIR, 2, SQH]
                "mask_t": np.ascontiguousarray(
                    m_t.reshape(NPAIR, 2, P, NPH, SQH).transpose(2, 3, 0, 1, 4)
                ),
            }
        )
    return in_maps


def run(inputs, trace=False):
    """Run the SPMD kernel; returns (output [B,S,D_K] f32, BassKernelResults)."""
    from concourse.bass_utils import run_bass_kernel_spmd

    nc = _get_nc()
    in_maps = _shard_inputs(**inputs)
    res = run_bass_kernel_spmd(
        nc, in_maps, core_ids=list(range(N_CORES)), trace=trace
    )
    out = np.empty((B, S, D_K), np.float32)
    for b in range(B):
        # each kv-half core returns partial oT [65, 2048]: rows 0-63 =
        # numerator.T, row 64 = partial softmax denominator
        oT = res.results[2 * b]["out"] + res.results[2 * b + 1]["out"]
        out[b] = (oT[0:D_K] / oT[D_K : D_K + 1]).T
    return out, res


def kernel(**inputs):
    out, _ = run(inputs, trace=False)
    return out
